# revision 6
# baseline (speedup 1.0000x reference)
"""2D Haar DWT (single-level) on Trainium2, 8-core data parallel.

Input  x: (8, 96, 512, 512) f32.
Output (LL, LH, HL, HH): each (8, 96, 256, 256) f32.

Math (stride-2 phase slices a=x[...,0::2,0::2], b=x[...,0::2,1::2],
c=x[...,1::2,0::2], d=x[...,1::2,1::2]):
    LL = 0.5*(a+b+c+d)   LH = 0.5*(a+b-c-d)
    HL = 0.5*(a-b+c-d)   HH = 0.5*(a-b-c+d)

Butterfly: ScalarE pre-scales odd rows (oh = 0.5*o), DVE row pass via
scalar_tensor_tensor (s = 0.5*e +/- oh), then column pass with plain
tensor_tensor over stride-2 views. (tensor_tensor_reduce would fuse the
scale, but its custom DVE-table op fails to execute on this runtime.)

Layout: one DMA per channel loads rows 4p..4p+3 into partition p (8 KiB
contiguous per partition). Even/odd rows are then strided views within
the tile. Output tiles store contiguously (partition p holds output rows
2p, 2p+1).

Sharding: core i gets batch i (8 batches / 8 cores, no communication).
"""

import numpy as np

import concourse.bacc as bacc
import concourse.mybir as mybir
from concourse.bass_utils import run_bass_kernel_spmd
from concourse.tile import TileContext

F32 = mybir.dt.float32
N_CORES = 8
C, H, W = 96, 512, 512
OH, OW = H // 2, W // 2
P = 128

_NC_CACHE = {}


def _build_nc(bufs=3):
    nc = bacc.Bacc()
    x = nc.declare_dram_parameter("x", [C, H, W], F32, isOutput=False)
    outs = {
        name: nc.declare_dram_parameter(name, [C, OH, OW], F32, isOutput=True)
        for name in ("ll", "lh", "hl", "hh")
    }
    ADD = mybir.AluOpType.add
    SUB = mybir.AluOpType.subtract
    MULT = mybir.AluOpType.mult

    with TileContext(nc) as tc:
        with (
            tc.tile_pool(name="pin", bufs=bufs) as pin,
            tc.tile_pool(name="pmid", bufs=bufs) as pmid,
            tc.tile_pool(name="pout", bufs=bufs) as pout,
        ):
            for ch in range(C):
                # partition p holds rows 4p..4p+3 of channel ch (contiguous)
                eo = pin.tile([P, 4 * W], F32, tag="eo")
                nc.sync.dma_start(
                    out=eo[:], in_=x[ch].rearrange("(p r) w -> p (r w)", p=P)
                )
                # row = 4p + 2*r2 + par  ->  ev[p, par, r2, w]
                ev = eo[:].rearrange("p (r2 par w) -> p par r2 w", r2=2, par=2)
                oh = pmid.tile([P, 2, W], F32, tag="oh")
                nc.scalar.mul(oh[:], ev[:, 1], 0.5)
                s = pmid.tile([P, 2, W], F32, tag="s")
                t = pmid.tile([P, 2, W], F32, tag="t")
                nc.vector.scalar_tensor_tensor(
                    out=s[:], in0=ev[:, 0], scalar=0.5, in1=oh[:], op0=MULT, op1=ADD
                )
                nc.vector.scalar_tensor_tensor(
                    out=t[:], in0=ev[:, 0], scalar=0.5, in1=oh[:], op0=MULT, op1=SUB
                )
                for name, src, op in (
                    ("ll", s, ADD),
                    ("lh", t, ADD),
                    ("hl", s, SUB),
                    ("hh", t, SUB),
                ):
                    ot = pout.tile([P, 2, OW], F32, tag="t_" + name)
                    nc.vector.tensor_tensor(
                        ot[:], src[:, :, 0::2], src[:, :, 1::2], op
                    )
                    # output row = 2p + r2
                    nc.sync.dma_start(
                        out=outs[name][ch].rearrange("(p r2) w -> p r2 w", p=P),
                        in_=ot[:],
                    )
    nc.finalize()
    return nc


def _get_nc():
    if "nc" not in _NC_CACHE:
        _NC_CACHE["nc"] = _build_nc()
    return _NC_CACHE["nc"]


def _run(x, trace=False):
    x = np.ascontiguousarray(np.asarray(x), dtype=np.float32)
    assert x.shape == (N_CORES, C, H, W), x.shape
    nc = _get_nc()
    in_maps = [{"x": x[i]} for i in range(N_CORES)]
    res = run_bass_kernel_spmd(nc, in_maps, core_ids=list(range(N_CORES)), trace=trace)
    out = tuple(
        np.stack([res.results[i][name] for i in range(N_CORES)])
        for name in ("ll", "lh", "hl", "hh")
    )
    return out, res


def kernel(x):
    out, _ = _run(x)
    return out


# revision 9
# speedup vs baseline: 139.8972x; 139.8972x over previous
"""2D Haar DWT (single-level) on Trainium2, 8-core data parallel.

Input  x: (8, 96, 512, 512) f32.
Output (LL, LH, HL, HH): each (8, 96, 256, 256) f32.

Math (stride-2 phase slices a=x[...,0::2,0::2], b=x[...,0::2,1::2],
c=x[...,1::2,0::2], d=x[...,1::2,1::2]):
    LL = 0.5*(a+b+c+d)   LH = 0.5*(a+b-c-d)
    HL = 0.5*(a-b+c-d)   HH = 0.5*(a-b-c+d)

Butterfly: ScalarE pre-scales odd rows (oh = 0.5*o), DVE row pass via
scalar_tensor_tensor (s = 0.5*e +/- oh), then column pass with plain
tensor_tensor over stride-2 views. (tensor_tensor_reduce would fuse the
scale, but its custom DVE-table op fails to execute on this runtime.)

Layout: one DMA per channel loads rows 4p..4p+3 into partition p (8 KiB
contiguous per partition). Even/odd rows are then strided views within
the tile. Output tiles store contiguously (partition p holds output rows
2p, 2p+1).

Sharding: core i gets batch i (8 batches / 8 cores, no communication).
"""

import numpy as np

import concourse.bacc as bacc
import concourse.mybir as mybir
from concourse.bass_utils import run_bass_kernel_spmd
from concourse.tile import TileContext

F32 = mybir.dt.float32
N_CORES = 8
C, H, W = 96, 512, 512
OH, OW = H // 2, W // 2
P = 128

_NC_CACHE = {}


def _build_nc(bufs=3, reps=1):
    """reps>1 repeats the whole body (idempotent) — used by test.py to
    measure true device time as a slope, since per-launch dispatch
    overhead through axon/PJRT is ~100 ms."""
    nc = bacc.Bacc()
    x = nc.declare_dram_parameter("x", [C, H, W], F32, isOutput=False)
    outs = {
        name: nc.declare_dram_parameter(name, [C, OH, OW], F32, isOutput=True)
        for name in ("ll", "lh", "hl", "hh")
    }
    ADD = mybir.AluOpType.add
    SUB = mybir.AluOpType.subtract
    MULT = mybir.AluOpType.mult

    with TileContext(nc) as tc:
        with (
            tc.tile_pool(name="pin", bufs=bufs) as pin,
            tc.tile_pool(name="pmid", bufs=bufs) as pmid,
            tc.tile_pool(name="pout", bufs=bufs) as pout,
        ):
            for ch in [c for _ in range(reps) for c in range(C)]:
                # partition p holds rows 4p..4p+3 of channel ch (contiguous)
                eo = pin.tile([P, 4 * W], F32, tag="eo")
                nc.sync.dma_start(
                    out=eo[:], in_=x[ch].rearrange("(p r) w -> p (r w)", p=P)
                )
                # row = 4p + 2*r2 + par  ->  ev[p, par, r2, w]
                ev = eo[:].rearrange("p (r2 par w) -> p par r2 w", r2=2, par=2)
                oh = pmid.tile([P, 2, W], F32, tag="oh")
                nc.scalar.mul(oh[:], ev[:, 1], 0.5)
                s = pmid.tile([P, 2, W], F32, tag="s")
                t = pmid.tile([P, 2, W], F32, tag="t")
                nc.vector.scalar_tensor_tensor(
                    out=s[:], in0=ev[:, 0], scalar=0.5, in1=oh[:], op0=MULT, op1=ADD
                )
                nc.vector.scalar_tensor_tensor(
                    out=t[:], in0=ev[:, 0], scalar=0.5, in1=oh[:], op0=MULT, op1=SUB
                )
                for name, src, op in (
                    ("ll", s, ADD),
                    ("lh", t, ADD),
                    ("hl", s, SUB),
                    ("hh", t, SUB),
                ):
                    ot = pout.tile([P, 2, OW], F32, tag="t_" + name)
                    nc.vector.tensor_tensor(
                        ot[:], src[:, :, 0::2], src[:, :, 1::2], op
                    )
                    # output row = 2p + r2; stores go out on ScalarE's HWDGE
                    # ring so loads (SP ring) and stores never queue behind
                    # each other
                    nc.scalar.dma_start(
                        out=outs[name][ch].rearrange("(p r2) w -> p r2 w", p=P),
                        in_=ot[:],
                    )
    nc.finalize()
    return nc


def _get_nc():
    if "nc" not in _NC_CACHE:
        _NC_CACHE["nc"] = _build_nc()
    return _NC_CACHE["nc"]


def _run(x, trace=False):
    x = np.ascontiguousarray(np.asarray(x), dtype=np.float32)
    assert x.shape == (N_CORES, C, H, W), x.shape
    nc = _get_nc()
    in_maps = [{"x": x[i]} for i in range(N_CORES)]
    res = run_bass_kernel_spmd(nc, in_maps, core_ids=list(range(N_CORES)), trace=trace)
    out = tuple(
        np.stack([res.results[i][name] for i in range(N_CORES)])
        for name in ("ll", "lh", "hl", "hh")
    )
    return out, res


def kernel(x):
    out, _ = _run(x)
    return out
